# revision 48
# baseline (speedup 1.0000x reference)
"""Dual-path self-attention (DPSA) kernel for 8 Trainium2 NeuronCores.

Reference computation (B=2, S=2048, D=1024, H=16, DK=64):
    Q_sem = X_sem @ Wq_sem + bq_sem   (per-head)
    K_sem = X_sem @ Wk_sem + bk_sem
    V     = X_sem @ Wv + bv
    Q_sal = X_sal @ Wq_sal + bq_sal
    K_sal = X_sal @ Wk_sal + bk_sal
    A = (Q_sem K_sem^T + gamma * Q_sal K_sal^T) / sqrt(DK)
    A = softmax(mask ? A : -1e9)
    out = (A V) @ Wo + bo

Sharding: batch x head-group. Core c handles batch c//4 and heads
[4*(c%4), 4*(c%4)+4). Each core computes its 4 heads' partial output
projection sum_h(O_h @ Wo[rows_h]); the host reduces the 4 partials per
batch and adds bo.

Device-side layout ("transposed attention"):
  - X^T [D, S] resident in SBUF (fp16); projections produce Q^T/K^T
    directly: psum[m=channels, n=seq] = Wchunk.T @ X^T chunk.
  - QcatT/KcatT per head: [128, S] with semantic rows 0:64, salience
    rows 64:128. One contract-128 matmul computes
    A^T = (Q_sem K_sem^T + gamma Q_sal K_sal^T)^T / sqrt(DK) fused at
    full PE efficiency (scales folded into Q-side weights on host).
  - exp on ScalarE (PSUM->SBUF, fp16 out); AV accumulation with
    lhsT = [V_h | ones] so the softmax denominator falls out as row 64
    of the AV psum for free.
  - Normalization delayed past AV: O_unnorm^T scaled by 1/denom (fast
    DVE reciprocal + fp32r PE partition-broadcast) before Wo.

Scheduling (v2):
  - AV matmuls run one kt-iteration behind their scores, so the PE's
    in-order stream never waits on ScalarE's exp of the same iteration.
  - Score psum rotates through 3 buffers (6 banks); Wo/norm psum shares
    the same tag; AV accumulators use the remaining 2 banks.
  - ScalarE does exp ONLY during attention (all psum->SBUF copies live
    on DVE); mt=1 projections / Wo tiles / normalization are emitted as
    filler units inside the attention kt loops to keep the PE fed while
    ScalarE paces the pipeline.
"""

import numpy as np

B, S, D, H = 2, 2048, 1024, 16
DK = D // H  # 64
N_CORES = 8
HG = 4  # head groups (cores per batch)
HPC = 4  # heads per core
DKC = HPC * DK  # 256 channels per core
QCHUNK = 512
NQC = S // QCHUNK  # 4
NKT = S // 128  # 16 key tiles
VSLOT = DK + 1  # V columns per (ktile, head) slot incl. ones column

_cached = {}


def _build_nc(with_qk_bias, with_v_bias):
    import concourse.bass as bass
    import concourse.tile as tile
    from concourse import bacc, mybir

    fp16 = mybir.dt.float16
    fp32 = mybir.dt.float32

    nc = bacc.Bacc(None)

    # ---- DRAM I/O (per-core shards) ----
    xt_sem_d = nc.dram_tensor("xt_sem", [D, S], fp16, kind="ExternalInput")
    xt_sal_d = nc.dram_tensor("xt_sal", [D, S], fp16, kind="ExternalInput")
    # weights pre-rearranged on host: w_r[p, c*C + j] = W[c*128 + p, j]
    wq_d = nc.dram_tensor("wq", [128, 8 * DKC], fp16, kind="ExternalInput")
    wk_d = nc.dram_tensor("wk", [128, 8 * DKC], fp16, kind="ExternalInput")
    wqs_d = nc.dram_tensor("wqs", [128, 8 * DKC], fp16, kind="ExternalInput")
    wks_d = nc.dram_tensor("wks", [128, 8 * DKC], fp16, kind="ExternalInput")
    wv_d = nc.dram_tensor("wv", [128, 8 * DKC], fp16, kind="ExternalInput")
    wo_d = nc.dram_tensor("wo", [128, 2 * D], fp16, kind="ExternalInput")
    if with_qk_bias:
        bqk_d = nc.dram_tensor("bqk", [1, 4 * DKC], fp16, kind="ExternalInput")
    if with_v_bias:
        bv_d = nc.dram_tensor("bv", [1, DKC], fp16, kind="ExternalInput")
    out_d = nc.dram_tensor("out", [S, D], fp16, kind="ExternalOutput")

    # denominator slot for head h: (partition row, region). Heads 0,1 in
    # region 0 at rows 0/32; heads 2,3 in region 1 at rows 0/32 — each
    # head-pair's reciprocal+normalize runs as soon as its AV finishes.
    # (matmul stationary operands require base partitions {0, 32, 64}.)
    def dloc(h):
        return (32 * (h % 2), h // 2)

    with tile.TileContext(nc) as tc:
        with (
            tc.tile_pool(name="persist", bufs=1) as persist,
            tc.tile_pool(name="psum", bufs=1, space="PSUM") as psum,
            tc.tile_pool(name="expp", bufs=5) as expp,
            tc.tile_pool(name="outp", bufs=3) as outp,
            tc.tile_pool(name="normp", bufs=2) as normp,
        ):
            qcat = persist.tile([128, HPC, S], fp16)
            kcat = persist.tile([128, HPC, S], fp16)
            v_sb = persist.tile([128, NKT, HPC, VSLOT], fp16)
            ones_sb = persist.tile([1, QCHUNK], fp16)
            onesT_sb = persist.tile([128, 64], fp16)
            wq_sb = persist.tile([128, 8 * DKC], fp16)
            wk_sb = persist.tile([128, 8 * DKC], fp16)
            wqs_sb = persist.tile([128, 8 * DKC], fp16)
            wks_sb = persist.tile([128, 8 * DKC], fp16)
            wv_sb = persist.tile([128, 8 * DKC], fp16)
            wo_sb = persist.tile([128, 2 * D], fp16)
            xt_sal = persist.tile([128, 8, S], fp16)
            xt_sem = persist.tile([128, 8, S], fp16)
            o_un = persist.tile([64, HPC, S], fp16)
            ot = persist.tile([128, 2, S], fp16)

            nc.vector.memset(ones_sb[:], 1.0)
            nc.vector.memset(onesT_sb[:], 1.0)
            nc.vector.memset(v_sb[:, :, :, DK : DK + 1], 1.0)

            # ---- input DMA: sync+scalar are HWDGE queues; gpsimd (slow
            # software DGE) only carries the two earliest-needed chunks.
            # Weights split in halves so the first matmuls start sooner.
            xt_sal_r = xt_sal_d.rearrange("(c p) s -> c p s", p=128)
            xt_sem_r = xt_sem_d.rearrange("(c p) s -> c p s", p=128)
            HB = 4 * DKC  # half the weight columns (kc blocks 0-3)

            # HW queues (sync/scalar) carry everything ordered by first
            # use; the slow gpsimd software queue gets only the two
            # latest-needed X_sem chunks.
            SH = S // 2
            QB = 2 * DKC  # quarter of the weight columns (kc blocks 0-1)
            nc.sync.dma_start(out=wqs_sb[:, :QB], in_=wqs_d[:, :QB])
            nc.scalar.dma_start(out=xt_sal[:, 0, :SH], in_=xt_sal_r[0][:, :SH])
            nc.gpsimd.dma_start(out=xt_sem[:, 6, :], in_=xt_sem_r[6])
            nc.sync.dma_start(out=wqs_sb[:, QB:HB], in_=wqs_d[:, QB:HB])
            nc.scalar.dma_start(out=xt_sal[:, 0, SH:], in_=xt_sal_r[0][:, SH:])
            nc.gpsimd.dma_start(out=xt_sem[:, 7, :], in_=xt_sem_r[7])
            nc.sync.dma_start(out=wks_sb[:, :QB], in_=wks_d[:, :QB])
            nc.scalar.dma_start(out=wqs_sb[:, HB:], in_=wqs_d[:, HB:])
            nc.sync.dma_start(out=xt_sal[:, 1, :SH], in_=xt_sal_r[1][:, :SH])
            nc.scalar.dma_start(out=xt_sal[:, 1, SH:], in_=xt_sal_r[1][:, SH:])
            nc.sync.dma_start(out=xt_sal[:, 2, :], in_=xt_sal_r[2])
            nc.scalar.dma_start(out=xt_sal[:, 3, :], in_=xt_sal_r[3])
            nc.sync.dma_start(out=xt_sal[:, 4, :], in_=xt_sal_r[4])
            nc.scalar.dma_start(out=xt_sal[:, 5, :], in_=xt_sal_r[5])
            nc.sync.dma_start(out=xt_sal[:, 6, :], in_=xt_sal_r[6])
            nc.scalar.dma_start(out=xt_sal[:, 7, :], in_=xt_sal_r[7])
            nc.sync.dma_start(out=wks_sb[:, QB:HB], in_=wks_d[:, QB:HB])
            nc.scalar.dma_start(out=wks_sb[:, HB:], in_=wks_d[:, HB:])
            nc.sync.dma_start(out=wq_sb[:, :HB], in_=wq_d[:, :HB])
            nc.scalar.dma_start(out=wq_sb[:, HB:], in_=wq_d[:, HB:])
            for kc in range(6):
                eng = nc.sync if kc % 2 == 0 else nc.scalar
                eng.dma_start(out=xt_sem[:, kc, :], in_=xt_sem_r[kc])
            nc.sync.dma_start(out=wk_sb[:, :HB], in_=wk_d[:, :HB])
            nc.scalar.dma_start(out=wk_sb[:, HB:], in_=wk_d[:, HB:])
            nc.sync.dma_start(out=wv_sb[:, :HB], in_=wv_d[:, :HB])
            nc.scalar.dma_start(out=wv_sb[:, HB:], in_=wv_d[:, HB:])
            nc.sync.dma_start(out=wo_sb[:], in_=wo_d[:])
            if with_qk_bias:
                bqk_sb = persist.tile([1, 4 * DKC], fp16)
                nc.sync.dma_start(out=bqk_sb[:], in_=bqk_d[:])
            if with_v_bias:
                bv_sb = persist.tile([1, DKC], fp16)
                nc.sync.dma_start(out=bv_sb[:], in_=bv_d[:])

            # ================= projections (Q/K first, then V) ============
            projs = [
                (wqs_sb, xt_sal, qcat, 64, 1),  # (W, X^T, dest, row0, bias-idx)
                (wks_sb, xt_sal, kcat, 64, 3),
                (wq_sb, xt_sem, qcat, 0, 0),
                (wk_sb, xt_sem, kcat, 0, 2),
            ]

            def _emit_proj_group(w_sb, x_sb, dest, row0, bidx, mt, nq0, cp2):
                """One psum group: two 512-query chunks of one mt-tile."""
                ps = psum.tile([128, 2 * QCHUNK], fp32, tag="sp", bufs=3)
                for j in range(2):
                    pv = ps[:, j * QCHUNK : (j + 1) * QCHUNK]
                    r_sl = slice((nq0 + j) * QCHUNK, (nq0 + j + 1) * QCHUNK)
                    for kc in range(8):
                        nc.tensor.matmul(
                            pv,
                            w_sb[:, kc * DKC + mt * 128 : kc * DKC + (mt + 1) * 128],
                            x_sb[:, kc, r_sl],
                            start=(kc == 0),
                            stop=(kc == 7 and not with_qk_bias),
                        )
                    if with_qk_bias:
                        nc.tensor.matmul(
                            pv,
                            bqk_sb[:, bidx * DKC + mt * 128 : bidx * DKC + (mt + 1) * 128],
                            ones_sb[:, :QCHUNK],
                            start=False,
                            stop=True,
                        )
                g_sl = slice(nq0 * QCHUNK, (nq0 + 2) * QCHUNK)
                nc.vector.tensor_copy(
                    dest[row0 : row0 + 64, 2 * mt, g_sl], ps[0:64, :]
                )
                cp2(dest[row0 : row0 + 64, 2 * mt + 1, g_sl], ps[64:128, :])

            # mt=0 for all projs (pre-attention; ScalarE idle -> split
            # the second copy onto it).
            # The two projections sharing an X are interleaved chunk-by-
            # chunk so the PE has work for every arriving X chunk instead
            # of draining one projection ahead of the DMA trickle.
            def _emit_proj_pair_mt0(pa, pb, nq0):
                tiles = []
                for _ in (pa, pb):
                    tiles.append(
                        psum.tile([128, 2 * QCHUNK], fp32, tag="sp", bufs=3,
                                  name="pp")
                    )
                for kc in range(8):
                    for (w_sb, x_sb, dest, row0, bidx), ps in zip((pa, pb), tiles):
                        for j in range(2):
                            r_sl = slice(
                                (nq0 + j) * QCHUNK, (nq0 + j + 1) * QCHUNK
                            )
                            nc.tensor.matmul(
                                ps[:, j * QCHUNK : (j + 1) * QCHUNK],
                                w_sb[:, kc * DKC : kc * DKC + 128],
                                x_sb[:, kc, r_sl],
                                start=(kc == 0),
                                stop=(kc == 7 and not with_qk_bias),
                            )
                for (w_sb, x_sb, dest, row0, bidx), ps in zip((pa, pb), tiles):
                    if with_qk_bias:
                        for j in range(2):
                            nc.tensor.matmul(
                                ps[:, j * QCHUNK : (j + 1) * QCHUNK],
                                bqk_sb[:, bidx * DKC : bidx * DKC + 128],
                                ones_sb[:, :QCHUNK],
                                start=False,
                                stop=True,
                            )
                    g_sl = slice(nq0 * QCHUNK, (nq0 + 2) * QCHUNK)
                    nc.vector.tensor_copy(
                        dest[row0 : row0 + 64, 0, g_sl], ps[0:64, :]
                    )
                    nc.scalar.copy(
                        dest[row0 : row0 + 64, 1, g_sl], ps[64:128, :]
                    )

            for pa, pb in ((projs[0], projs[1]), (projs[2], projs[3])):
                for nq0 in (0, 2):
                    _emit_proj_pair_mt0(pa, pb, nq0)

            # V: natural layout [s, dv]; two s-tiles per psum group
            for st2 in range(NKT // 2):
                ps = psum.tile([128, 2 * QCHUNK], fp32, tag="sp", bufs=3)
                for j in range(2):
                    st = st2 * 2 + j
                    vp = ps[:, j * DKC : (j + 1) * DKC]
                    for kc in range(8):
                        nc.tensor.matmul(
                            vp,
                            xt_sem[:, kc, st * 128 : (st + 1) * 128],
                            wv_sb[:, kc * DKC : (kc + 1) * DKC],
                            start=(kc == 0),
                            stop=(kc == 7 and not with_v_bias),
                        )
                    if with_v_bias:
                        nc.tensor.matmul(
                            vp, ones_sb[:, :128], bv_sb[:], start=False, stop=True
                        )
                vcp = nc.vector.tensor_copy if st2 % 2 == 0 else nc.scalar.copy
                vcp(
                    v_sb[:, st2 * 2 : st2 * 2 + 2, :, 0:DK],
                    ps[:, : 2 * DKC].rearrange("p (t h d) -> p t h d", t=2, h=HPC),
                )

            # Wo for one 128-row output tile (psum shares the sp tag)
            def _emit_wo(st):
                # final-qc tiles copy via the idle ScalarE at the tail
                cp = nc.scalar.copy if st >= (NQC - 1) * 4 else nc.vector.tensor_copy
                ob = outp.tile([128, D], fp16)
                for nh in range(2):
                    wp = psum.tile([128, 512], fp32, tag="sp", bufs=3, name="wp")
                    for cc in range(2):
                        nc.tensor.matmul(
                            wp[:],
                            ot[:, cc, st * 128 : (st + 1) * 128],
                            wo_sb[:, cc * D + nh * 512 : cc * D + (nh + 1) * 512],
                            start=(cc == 0),
                            stop=(cc == 1),
                        )
                    cp(ob[:, nh * 512 : (nh + 1) * 512], wp[:])
                nc.sync.dma_start(out=out_d[st * 128 : (st + 1) * 128, :], in_=ob[:])

            # normalization for one head-pair of one qc: 1/denom (fp16)
            # broadcast across partitions on the PE (full-rate fp16 matmul),
            # then fused multiply on DVE
            def _emit_norm(qc, hp, r16):
                q_sl = slice(qc * QCHUNK, (qc + 1) * QCHUNK)
                for h in (hp, hp + 1):
                    dr, reg = dloc(h)
                    bc = psum.tile([64, QCHUNK], fp32, tag="sp", bufs=3, name="bc")
                    nc.tensor.matmul(
                        bc,
                        onesT_sb[dr : dr + 1, :],
                        r16[dr : dr + 1, reg, :],
                        start=True,
                        stop=True,
                    )
                    nc.vector.tensor_tensor(
                        ot[(h % 2) * 64 : (h % 2) * 64 + 64, h // 2, q_sl],
                        o_un[:, h, q_sl],
                        bc[:],
                        mybir.AluOpType.mult,
                    )

            # ===== attention =====
            # Filler units: closures emitting PE-side work, popped inside
            # the kt loops so the PE stays fed while ScalarE paces exp.
            filler_q = []

            def pop_filler():
                if filler_q:
                    filler_q.pop(0)()

            # mt=1 projections (heads 2,3) are the filler for (qc0, hp0)
            for _p in projs:
                for nq0 in (0, 2):
                    w_sb_, x_sb_, dest_, row0_, bidx_ = _p
                    filler_q.append(
                        lambda w=w_sb_, x=x_sb_, d=dest_, r=row0_, b=bidx_, n=nq0: (
                            _emit_proj_group(w, x, d, r, b, mt=1, nq0=n,
                                             cp2=nc.vector.tensor_copy)
                        )
                    )

            for qc in range(NQC):
                q_sl = slice(qc * QCHUNK, (qc + 1) * QCHUNK)
                dn = normp.tile([65, 2, QCHUNK], fp32, tag="dn", bufs=2, name="dn")
                rc = normp.tile([65, 2, QCHUNK], fp32, tag="rc", bufs=2, name="rc")
                r16 = normp.tile([65, 2, QCHUNK], fp16, tag="r16", bufs=2, name="r16")
                for hp in (0, 2):
                    avs = {}
                    for h in (hp, hp + 1):
                        avs[h] = psum.tile(
                            [65, QCHUNK], fp32, tag="av", name=f"av{h}", bufs=2
                        )
                    first = {hp: True, hp + 1: True}
                    pend = []

                    def _emit_av(batch):
                        for h_, et_, pkt_ in batch:
                            for g in range(2):
                                nc.tensor.matmul(
                                    avs[h_],
                                    v_sb[:, pkt_ + g, h_, :],
                                    et_[:, g * QCHUNK : (g + 1) * QCHUNK],
                                    start=first[h_],
                                    stop=(pkt_ + g == NKT - 1),
                                )
                                first[h_] = False

                    for it, kt in enumerate(range(0, NKT, 2)):
                        cur = []
                        for h in (hp, hp + 1):
                            sp = psum.tile([128, 2 * QCHUNK], fp32, tag="sp", bufs=3)
                            for g in range(2):
                                nc.tensor.matmul(
                                    sp[:, g * QCHUNK : (g + 1) * QCHUNK],
                                    kcat[:, h, (kt + g) * 128 : (kt + g + 1) * 128],
                                    qcat[:, h, q_sl],
                                    start=True,
                                    stop=True,
                                )
                            et = expp.tile([128, 2 * QCHUNK], fp16)
                            nc.scalar.activation(
                                et[:], sp[:], mybir.ActivationFunctionType.Exp
                            )
                            cur.append((h, et, kt))
                        # filler first: it runs while the previous
                        # iteration's exp drains, then AV (deps long done)
                        if (qc == 0 and hp == 0) or it % 2 == 1:
                            pop_filler()
                        _emit_av(pend)
                        pend = cur
                    _emit_av(pend)

                    # Last hp: the big copies go to the (by now idle)
                    # ScalarE so the tail's DVE chain is shorter.
                    last = qc == NQC - 1 and hp == 2
                    ocp = nc.scalar.copy if last else nc.vector.tensor_copy
                    reg = hp // 2
                    for h in (hp, hp + 1):
                        dr, _ = dloc(h)
                        ocp(o_un[:, h, q_sl], avs[h][0:64, :])
                        nc.vector.tensor_copy(
                            dn[dr : dr + 1, reg, :], avs[h][64:65, :]
                        )
                    nc.vector.reciprocal_approx_fast(
                        out=rc[0:65, reg, :], in_=dn[0:65, reg, :]
                    )
                    rcp = nc.scalar.copy if last else nc.vector.tensor_copy
                    rcp(r16[0:65, reg, :], rc[0:65, reg, :])
                    filler_q.append(
                        lambda q=qc, p=hp, r=r16: _emit_norm(q, p, r)
                    )

                # spacer so the first Wo pop lands well after its ot deps
                filler_q.append(lambda: None)
                for st in range(qc * 4, qc * 4 + 4):
                    filler_q.append(lambda s=st: _emit_wo(s))

            while filler_q:
                filler_q.pop(0)()

    nc.compile()
    return nc


def _get_nc(key):
    if key not in _cached:
        _cached[key] = _build_nc(*key)
    return _cached[key]


def _host_reference(X_sem, X_sal, mask, Wq_sem, bq_sem, Wk_sem, bk_sem, Wv,
                    bv, Wq_sal, bq_sal, Wk_sal, bk_sal, Wo, bo, gamma):
    f32 = np.float32
    scale = f32(1.0 / np.sqrt(DK))

    def heads(x):
        return x.reshape(B, S, H, DK).transpose(0, 2, 1, 3)

    def lin(x, W, b):
        return (x.reshape(B * S, D) @ np.asarray(W, f32)).reshape(B, S, D) + np.asarray(b, f32)

    Xm = np.asarray(X_sem, f32)
    Xl = np.asarray(X_sal, f32)
    Q = heads(lin(Xm, Wq_sem, bq_sem))
    K = heads(lin(Xm, Wk_sem, bk_sem))
    V = heads(lin(Xm, Wv, bv))
    Ql = heads(lin(Xl, Wq_sal, bq_sal))
    Kl = heads(lin(Xl, Wk_sal, bk_sal))
    out = np.empty((B, S, D), f32)
    for b in range(B):
        for h in range(H):
            A = (Q[b, h] @ K[b, h].T + gamma * (Ql[b, h] @ Kl[b, h].T)) * scale
            A = np.where(np.asarray(mask)[b, 0] == 0, f32(-1e9), A)
            A -= A.max(axis=-1, keepdims=True)
            np.exp(A, out=A)
            A /= A.sum(axis=-1, keepdims=True)
            out[b, :, h * DK : (h + 1) * DK] = A @ V[b, h]
    y = out.reshape(B * S, D) @ np.asarray(Wo, f32)
    return (y + np.asarray(bo, f32)).reshape(B, S, D)


def _rearrange_w(w):
    # [1024, C] -> [128, 8*C] with w_r[p, c*C + j] = w[c*128 + p, j]
    C = w.shape[1]
    return np.ascontiguousarray(
        w.reshape(8, 128, C).transpose(1, 0, 2).reshape(128, 8 * C)
    )


def _run_spmd_fast(nc, in_maps, n_cores):
    """run_bass_via_pjrt's multi-core path, but downloading each output
    array once instead of once per core (the stock helper re-gathers the
    sharded global for every core slice -- ~0.3s x 8 over the tunnel)."""
    import jax
    import numpy as _np
    from jax.sharding import Mesh, PartitionSpec
    from jax.experimental.shard_map import shard_map
    from concourse import mybir
    from concourse.bass2jax import (
        _bass_exec_p,
        install_neuronx_cc_hook,
        partition_id_tensor,
    )

    install_neuronx_cc_hook()
    partition_name = nc.partition_id_tensor.name if nc.partition_id_tensor else None
    in_names, out_names, out_avals, zero_outs = [], [], [], []
    for alloc in nc.m.functions[0].allocations:
        if not isinstance(alloc, mybir.MemoryLocationSet):
            continue
        name = alloc.memorylocations[0].name
        if alloc.kind == "ExternalInput":
            if name != partition_name:
                in_names.append(name)
        elif alloc.kind == "ExternalOutput":
            dt = mybir.dt.np(alloc.dtype)
            out_names.append(name)
            out_avals.append(jax.core.ShapedArray(tuple(alloc.tensor_shape), dt))
            zero_outs.append(_np.zeros(tuple(alloc.tensor_shape), dt))
    n_params, n_outs = len(in_names), len(out_names)
    in_names = in_names + out_names + ([partition_name] if partition_name else [])
    donate = tuple(range(n_params, n_params + n_outs))

    def _body(*args):
        operands = list(args)
        if partition_name is not None:
            operands.append(partition_id_tensor())
        return tuple(_bass_exec_p.bind(
            *operands,
            out_avals=tuple(out_avals),
            in_names=tuple(in_names),
            out_names=tuple(out_names),
            lowering_input_output_aliases=(),
            sim_require_finite=True,
            sim_require_nnan=True,
            nc=nc,
        ))

    devices = jax.devices()[:n_cores]
    mesh = Mesh(_np.asarray(devices), ("core",))
    sharded = jax.jit(
        shard_map(_body, mesh=mesh, in_specs=(PartitionSpec("core"),) * (n_params + n_outs),
                  out_specs=(PartitionSpec("core"),) * n_outs, check_rep=False),
        donate_argnums=donate, keep_unused=True,
    )
    concat_in = [
        _np.concatenate([m[in_names[i]] for m in in_maps], axis=0)
        for i in range(n_params)
    ]
    concat_zeros = [
        _np.zeros((n_cores * z.shape[0], *z.shape[1:]), z.dtype) for z in zero_outs
    ]
    out_arrs = sharded(*concat_in, *concat_zeros)
    gathered = [
        _np.asarray(a).reshape(n_cores, *out_avals[i].shape)
        for i, a in enumerate(out_arrs)
    ]
    return [
        {name: gathered[i][c] for i, name in enumerate(out_names)}
        for c in range(n_cores)
    ]


def kernel(X_sem, X_sal, mask, Wq_sem, bq_sem, Wk_sem, bk_sem, Wv, bv,
           Wq_sal, bq_sal, Wk_sal, bk_sal, Wo, bo, gamma):
    from concourse.bass_utils import run_bass_kernel_spmd

    X_sem = np.asarray(X_sem)
    X_sal = np.asarray(X_sal)
    mask = np.asarray(mask)
    f32 = np.float32
    scale = f32(1.0 / np.sqrt(DK))
    g = f32(np.asarray(gamma).reshape(()))

    wq_full = (np.asarray(Wq_sem) * scale).astype(np.float16)
    bq_full = (np.asarray(bq_sem) * scale).astype(np.float16)
    wqs_full = (np.asarray(Wq_sal) * (g * scale)).astype(np.float16)
    bqs_full = (np.asarray(bq_sal) * (g * scale)).astype(np.float16)
    wk_full = np.asarray(Wk_sem).astype(np.float16)
    bk_full = np.asarray(bk_sem).astype(np.float16)
    wks_full = np.asarray(Wk_sal).astype(np.float16)
    bks_full = np.asarray(bk_sal).astype(np.float16)
    wv_full = np.asarray(Wv).astype(np.float16)
    bv_full = np.asarray(bv).astype(np.float16)
    wo_full = np.asarray(Wo).astype(np.float16)

    if not bool(np.all(mask)):
        # Masks with zeros never occur in this problem's input spec
        # (fill: ones); handle them exactly via a host fallback.
        return _host_reference(
            X_sem, X_sal, mask, Wq_sem, bq_sem, Wk_sem, bk_sem, Wv, bv,
            Wq_sal, bq_sal, Wk_sal, bk_sal, Wo, bo, g,
        )

    with_qk_bias = bool(
        np.any(np.asarray(bq_sem)) or np.any(np.asarray(bq_sal))
        or np.any(np.asarray(bk_sem)) or np.any(np.asarray(bk_sal))
    )
    with_v_bias = bool(np.any(np.asarray(bv)))

    nc = _get_nc((with_qk_bias, with_v_bias))

    xt = []
    for b in range(B):
        xt.append((
            np.ascontiguousarray(X_sem[b].T.astype(np.float16)),
            np.ascontiguousarray(X_sal[b].T.astype(np.float16)),
        ))

    in_maps = []
    for c in range(N_CORES):
        b, hg = c // HG, c % HG
        blk = slice(hg * DKC, (hg + 1) * DKC)
        m = {
            "xt_sem": xt[b][0],
            "xt_sal": xt[b][1],
            "wq": _rearrange_w(wq_full[:, blk]),
            "wk": _rearrange_w(wk_full[:, blk]),
            "wqs": _rearrange_w(wqs_full[:, blk]),
            "wks": _rearrange_w(wks_full[:, blk]),
            "wv": _rearrange_w(wv_full[:, blk]),
            "wo": np.ascontiguousarray(
                wo_full[blk].reshape(2, 128, D).transpose(1, 0, 2).reshape(128, 2 * D)
            ),
        }
        if with_qk_bias:
            m["bqk"] = np.concatenate(
                [bq_full[blk], bqs_full[blk], bk_full[blk], bks_full[blk]]
            ).reshape(1, 4 * DKC)
        if with_v_bias:
            m["bv"] = bv_full[blk].reshape(1, DKC)
        in_maps.append(m)

    try:
        results = _run_spmd_fast(nc, in_maps, N_CORES)
    except Exception:
        results = run_bass_kernel_spmd(
            nc, in_maps, core_ids=list(range(N_CORES))
        ).results

    out = np.zeros((B, S, D), dtype=f32)
    for c in range(N_CORES):
        out[c // HG] += results[c]["out"].astype(f32)
    out += np.asarray(bo).astype(f32)
    return out


# revision 50
# speedup vs baseline: 1.0122x; 1.0122x over previous
"""Dual-path self-attention (DPSA) kernel for 8 Trainium2 NeuronCores.

Reference computation (B=2, S=2048, D=1024, H=16, DK=64):
    Q_sem = X_sem @ Wq_sem + bq_sem   (per-head)
    K_sem = X_sem @ Wk_sem + bk_sem
    V     = X_sem @ Wv + bv
    Q_sal = X_sal @ Wq_sal + bq_sal
    K_sal = X_sal @ Wk_sal + bk_sal
    A = (Q_sem K_sem^T + gamma * Q_sal K_sal^T) / sqrt(DK)
    A = softmax(mask ? A : -1e9)
    out = (A V) @ Wo + bo

Sharding: batch x head-group. Core c handles batch c//4 and heads
[4*(c%4), 4*(c%4)+4). Each core computes its 4 heads' partial output
projection sum_h(O_h @ Wo[rows_h]); the host reduces the 4 partials per
batch and adds bo.

Device-side layout ("transposed attention"):
  - X^T [D, S] resident in SBUF (fp16); projections produce Q^T/K^T
    directly: psum[m=channels, n=seq] = Wchunk.T @ X^T chunk.
  - QcatT/KcatT per head: [128, S] with semantic rows 0:64, salience
    rows 64:128. One contract-128 matmul computes
    A^T = (Q_sem K_sem^T + gamma Q_sal K_sal^T)^T / sqrt(DK) fused at
    full PE efficiency (scales folded into Q-side weights on host).
  - exp on ScalarE (PSUM->SBUF, fp16 out); AV accumulation with
    lhsT = [V_h | ones] so the softmax denominator falls out as row 64
    of the AV psum for free.
  - Normalization delayed past AV: O_unnorm^T scaled by 1/denom (fast
    DVE reciprocal + fp32r PE partition-broadcast) before Wo.

Scheduling (v2):
  - AV matmuls run one kt-iteration behind their scores, so the PE's
    in-order stream never waits on ScalarE's exp of the same iteration.
  - Score psum rotates through 3 buffers (6 banks); Wo/norm psum shares
    the same tag; AV accumulators use the remaining 2 banks.
  - ScalarE does exp ONLY during attention (all psum->SBUF copies live
    on DVE); mt=1 projections / Wo tiles / normalization are emitted as
    filler units inside the attention kt loops to keep the PE fed while
    ScalarE paces the pipeline.
"""

import numpy as np

B, S, D, H = 2, 2048, 1024, 16
DK = D // H  # 64
N_CORES = 8
HG = 4  # head groups (cores per batch)
HPC = 4  # heads per core
DKC = HPC * DK  # 256 channels per core
QCHUNK = 512
NQC = S // QCHUNK  # 4
NKT = S // 128  # 16 key tiles
VSLOT = DK + 1  # V columns per (ktile, head) slot incl. ones column

_cached = {}


def _build_nc(with_qk_bias, with_v_bias):
    import concourse.bass as bass
    import concourse.tile as tile
    from concourse import bacc, mybir

    fp16 = mybir.dt.float16
    fp32 = mybir.dt.float32

    nc = bacc.Bacc(None)

    # ---- DRAM I/O (per-core shards) ----
    xt_sem_d = nc.dram_tensor("xt_sem", [D, S], fp16, kind="ExternalInput")
    xt_sal_d = nc.dram_tensor("xt_sal", [D, S], fp16, kind="ExternalInput")
    # weights pre-rearranged on host: w_r[p, c*C + j] = W[c*128 + p, j]
    wq_d = nc.dram_tensor("wq", [128, 8 * DKC], fp16, kind="ExternalInput")
    wk_d = nc.dram_tensor("wk", [128, 8 * DKC], fp16, kind="ExternalInput")
    wqs_d = nc.dram_tensor("wqs", [128, 8 * DKC], fp16, kind="ExternalInput")
    wks_d = nc.dram_tensor("wks", [128, 8 * DKC], fp16, kind="ExternalInput")
    wv_d = nc.dram_tensor("wv", [128, 8 * DKC], fp16, kind="ExternalInput")
    wo_d = nc.dram_tensor("wo", [128, 2 * D], fp16, kind="ExternalInput")
    if with_qk_bias:
        bqk_d = nc.dram_tensor("bqk", [1, 4 * DKC], fp16, kind="ExternalInput")
    if with_v_bias:
        bv_d = nc.dram_tensor("bv", [1, DKC], fp16, kind="ExternalInput")
    out_d = nc.dram_tensor("out", [S, D], fp16, kind="ExternalOutput")

    # denominator slot for head h: (partition row, region). Heads 0,1 in
    # region 0 at rows 0/32; heads 2,3 in region 1 at rows 0/32 — each
    # head-pair's reciprocal+normalize runs as soon as its AV finishes.
    # (matmul stationary operands require base partitions {0, 32, 64}.)
    def dloc(h):
        return (32 * (h % 2), h // 2)

    with tile.TileContext(nc) as tc:
        with (
            tc.tile_pool(name="persist", bufs=1) as persist,
            tc.tile_pool(name="psum", bufs=1, space="PSUM") as psum,
            tc.tile_pool(name="expp", bufs=5) as expp,
            tc.tile_pool(name="outp", bufs=3) as outp,
            tc.tile_pool(name="normp", bufs=2) as normp,
        ):
            qcat = persist.tile([128, HPC, S], fp16)
            kcat = persist.tile([128, HPC, S], fp16)
            v_sb = persist.tile([128, NKT, HPC, VSLOT], fp16)
            ones_sb = persist.tile([1, QCHUNK], fp16)
            onesT_sb = persist.tile([128, 64], fp16)
            wq_sb = persist.tile([128, 8 * DKC], fp16)
            wk_sb = persist.tile([128, 8 * DKC], fp16)
            wqs_sb = persist.tile([128, 8 * DKC], fp16)
            wks_sb = persist.tile([128, 8 * DKC], fp16)
            wv_sb = persist.tile([128, 8 * DKC], fp16)
            wo_sb = persist.tile([128, 2 * D], fp16)
            xt_sal = persist.tile([128, 8, S], fp16)
            xt_sem = persist.tile([128, 8, S], fp16)
            o_un = persist.tile([64, HPC, S], fp16)
            ot = persist.tile([128, 2, S], fp16)

            nc.vector.memset(ones_sb[:], 1.0)
            nc.vector.memset(onesT_sb[:], 1.0)
            nc.vector.memset(v_sb[:, :, :, DK : DK + 1], 1.0)

            # ---- input DMA: sync+scalar are HWDGE queues; gpsimd (slow
            # software DGE) only carries the two earliest-needed chunks.
            # Weights split in halves so the first matmuls start sooner.
            xt_sal_r = xt_sal_d.rearrange("(c p) s -> c p s", p=128)
            xt_sem_r = xt_sem_d.rearrange("(c p) s -> c p s", p=128)
            HB = 4 * DKC  # half the weight columns (kc blocks 0-3)

            # HW queues (sync/scalar) carry everything ordered by first
            # use; the slow gpsimd software queue gets only the two
            # latest-needed X_sem chunks.
            SH = S // 2
            nc.sync.dma_start(out=wqs_sb[:, :HB], in_=wqs_d[:, :HB])
            nc.scalar.dma_start(out=wqs_sb[:, HB:], in_=wqs_d[:, HB:])
            nc.gpsimd.dma_start(out=xt_sem[:, 6, :], in_=xt_sem_r[6])
            nc.sync.dma_start(out=xt_sal[:, 0, :SH], in_=xt_sal_r[0][:, :SH])
            nc.scalar.dma_start(out=xt_sal[:, 0, SH:], in_=xt_sal_r[0][:, SH:])
            nc.gpsimd.dma_start(out=xt_sem[:, 7, :], in_=xt_sem_r[7])
            nc.sync.dma_start(out=xt_sal[:, 1, :SH], in_=xt_sal_r[1][:, :SH])
            nc.scalar.dma_start(out=xt_sal[:, 1, SH:], in_=xt_sal_r[1][:, SH:])
            nc.sync.dma_start(out=xt_sal[:, 2, :], in_=xt_sal_r[2])
            nc.scalar.dma_start(out=xt_sal[:, 3, :], in_=xt_sal_r[3])
            nc.sync.dma_start(out=xt_sal[:, 4, :], in_=xt_sal_r[4])
            nc.scalar.dma_start(out=xt_sal[:, 5, :], in_=xt_sal_r[5])
            nc.sync.dma_start(out=xt_sal[:, 6, :], in_=xt_sal_r[6])
            nc.scalar.dma_start(out=xt_sal[:, 7, :], in_=xt_sal_r[7])
            nc.sync.dma_start(out=wks_sb[:, :HB], in_=wks_d[:, :HB])
            nc.scalar.dma_start(out=wks_sb[:, HB:], in_=wks_d[:, HB:])
            nc.sync.dma_start(out=wq_sb[:, :HB], in_=wq_d[:, :HB])
            nc.scalar.dma_start(out=wq_sb[:, HB:], in_=wq_d[:, HB:])
            for kc in range(6):
                eng = nc.sync if kc % 2 == 0 else nc.scalar
                eng.dma_start(out=xt_sem[:, kc, :], in_=xt_sem_r[kc])
            nc.sync.dma_start(out=wk_sb[:, :HB], in_=wk_d[:, :HB])
            nc.scalar.dma_start(out=wk_sb[:, HB:], in_=wk_d[:, HB:])
            nc.sync.dma_start(out=wv_sb[:, :HB], in_=wv_d[:, :HB])
            nc.scalar.dma_start(out=wv_sb[:, HB:], in_=wv_d[:, HB:])
            nc.sync.dma_start(out=wo_sb[:], in_=wo_d[:])
            if with_qk_bias:
                bqk_sb = persist.tile([1, 4 * DKC], fp16)
                nc.sync.dma_start(out=bqk_sb[:], in_=bqk_d[:])
            if with_v_bias:
                bv_sb = persist.tile([1, DKC], fp16)
                nc.sync.dma_start(out=bv_sb[:], in_=bv_d[:])

            # ================= projections (Q/K first, then V) ============
            projs = [
                (wqs_sb, xt_sal, qcat, 64, 1),  # (W, X^T, dest, row0, bias-idx)
                (wks_sb, xt_sal, kcat, 64, 3),
                (wq_sb, xt_sem, qcat, 0, 0),
                (wk_sb, xt_sem, kcat, 0, 2),
            ]

            def _emit_proj_group(w_sb, x_sb, dest, row0, bidx, mt, nq0, cp2):
                """One psum group: two 512-query chunks of one mt-tile."""
                ps = psum.tile([128, 2 * QCHUNK], fp32, tag="sp", bufs=3)
                for j in range(2):
                    pv = ps[:, j * QCHUNK : (j + 1) * QCHUNK]
                    r_sl = slice((nq0 + j) * QCHUNK, (nq0 + j + 1) * QCHUNK)
                    for kc in range(8):
                        nc.tensor.matmul(
                            pv,
                            w_sb[:, kc * DKC + mt * 128 : kc * DKC + (mt + 1) * 128],
                            x_sb[:, kc, r_sl],
                            start=(kc == 0),
                            stop=(kc == 7 and not with_qk_bias),
                        )
                    if with_qk_bias:
                        nc.tensor.matmul(
                            pv,
                            bqk_sb[:, bidx * DKC + mt * 128 : bidx * DKC + (mt + 1) * 128],
                            ones_sb[:, :QCHUNK],
                            start=False,
                            stop=True,
                        )
                g_sl = slice(nq0 * QCHUNK, (nq0 + 2) * QCHUNK)
                nc.vector.tensor_copy(
                    dest[row0 : row0 + 64, 2 * mt, g_sl], ps[0:64, :]
                )
                cp2(dest[row0 : row0 + 64, 2 * mt + 1, g_sl], ps[64:128, :])

            # mt=0 for all projs (pre-attention; ScalarE idle -> split
            # the second copy onto it)
            for _p in projs:
                for nq0 in (0, 2):
                    _emit_proj_group(*_p, mt=0, nq0=nq0, cp2=nc.scalar.copy)

            # V: natural layout [s, dv]; two s-tiles per psum group
            for st2 in range(NKT // 2):
                ps = psum.tile([128, 2 * QCHUNK], fp32, tag="sp", bufs=3)
                for j in range(2):
                    st = st2 * 2 + j
                    vp = ps[:, j * DKC : (j + 1) * DKC]
                    for kc in range(8):
                        nc.tensor.matmul(
                            vp,
                            xt_sem[:, kc, st * 128 : (st + 1) * 128],
                            wv_sb[:, kc * DKC : (kc + 1) * DKC],
                            start=(kc == 0),
                            stop=(kc == 7 and not with_v_bias),
                        )
                    if with_v_bias:
                        nc.tensor.matmul(
                            vp, ones_sb[:, :128], bv_sb[:], start=False, stop=True
                        )
                vcp = nc.vector.tensor_copy if st2 % 2 == 0 else nc.scalar.copy
                vcp(
                    v_sb[:, st2 * 2 : st2 * 2 + 2, :, 0:DK],
                    ps[:, : 2 * DKC].rearrange("p (t h d) -> p t h d", t=2, h=HPC),
                )

            # Wo for one 128-row output tile (psum shares the sp tag)
            def _emit_wo(st):
                # final-qc tiles copy via the idle ScalarE at the tail
                cp = nc.scalar.copy if st >= (NQC - 1) * 4 else nc.vector.tensor_copy
                ob = outp.tile([128, D], fp16)
                for nh in range(2):
                    wp = psum.tile([128, 512], fp32, tag="sp", bufs=3, name="wp")
                    for cc in range(2):
                        nc.tensor.matmul(
                            wp[:],
                            ot[:, cc, st * 128 : (st + 1) * 128],
                            wo_sb[:, cc * D + nh * 512 : cc * D + (nh + 1) * 512],
                            start=(cc == 0),
                            stop=(cc == 1),
                        )
                    cp(ob[:, nh * 512 : (nh + 1) * 512], wp[:])
                nc.sync.dma_start(out=out_d[st * 128 : (st + 1) * 128, :], in_=ob[:])

            # normalization for one head-pair of one qc: 1/denom (fp16)
            # broadcast across partitions on the PE (full-rate fp16 matmul),
            # then fused multiply on DVE
            def _emit_norm(qc, hp, r16):
                q_sl = slice(qc * QCHUNK, (qc + 1) * QCHUNK)
                for h in (hp, hp + 1):
                    dr, reg = dloc(h)
                    bc = psum.tile([64, QCHUNK], fp32, tag="sp", bufs=3, name="bc")
                    nc.tensor.matmul(
                        bc,
                        onesT_sb[dr : dr + 1, :],
                        r16[dr : dr + 1, reg, :],
                        start=True,
                        stop=True,
                    )
                    nc.vector.tensor_tensor(
                        ot[(h % 2) * 64 : (h % 2) * 64 + 64, h // 2, q_sl],
                        o_un[:, h, q_sl],
                        bc[:],
                        mybir.AluOpType.mult,
                    )

            # ===== attention =====
            # Filler units: closures emitting PE-side work, popped inside
            # the kt loops so the PE stays fed while ScalarE paces exp.
            filler_q = []

            def pop_filler():
                if filler_q:
                    filler_q.pop(0)()

            # mt=1 projections (heads 2,3) are the filler for (qc0, hp0)
            for _p in projs:
                for nq0 in (0, 2):
                    w_sb_, x_sb_, dest_, row0_, bidx_ = _p
                    filler_q.append(
                        lambda w=w_sb_, x=x_sb_, d=dest_, r=row0_, b=bidx_, n=nq0: (
                            _emit_proj_group(w, x, d, r, b, mt=1, nq0=n,
                                             cp2=nc.vector.tensor_copy)
                        )
                    )

            for qc in range(NQC):
                q_sl = slice(qc * QCHUNK, (qc + 1) * QCHUNK)
                dn = normp.tile([65, 2, QCHUNK], fp32, tag="dn", bufs=2, name="dn")
                rc = normp.tile([65, 2, QCHUNK], fp32, tag="rc", bufs=2, name="rc")
                r16 = normp.tile([65, 2, QCHUNK], fp16, tag="r16", bufs=2, name="r16")
                for hp in (0, 2):
                    avs = {}
                    for h in (hp, hp + 1):
                        avs[h] = psum.tile(
                            [65, QCHUNK], fp32, tag="av", name=f"av{h}", bufs=2
                        )
                    first = {hp: True, hp + 1: True}
                    pend = []

                    def _emit_av(batch):
                        for h_, et_, pkt_ in batch:
                            for g in range(2):
                                nc.tensor.matmul(
                                    avs[h_],
                                    v_sb[:, pkt_ + g, h_, :],
                                    et_[:, g * QCHUNK : (g + 1) * QCHUNK],
                                    start=first[h_],
                                    stop=(pkt_ + g == NKT - 1),
                                )
                                first[h_] = False

                    for it, kt in enumerate(range(0, NKT, 2)):
                        cur = []
                        for h in (hp, hp + 1):
                            sp = psum.tile([128, 2 * QCHUNK], fp32, tag="sp", bufs=3)
                            for g in range(2):
                                nc.tensor.matmul(
                                    sp[:, g * QCHUNK : (g + 1) * QCHUNK],
                                    kcat[:, h, (kt + g) * 128 : (kt + g + 1) * 128],
                                    qcat[:, h, q_sl],
                                    start=True,
                                    stop=True,
                                )
                            et = expp.tile([128, 2 * QCHUNK], fp16)
                            nc.scalar.activation(
                                et[:], sp[:], mybir.ActivationFunctionType.Exp
                            )
                            cur.append((h, et, kt))
                        # filler first: it runs while the previous
                        # iteration's exp drains, then AV (deps long done)
                        if (qc == 0 and hp == 0) or it % 2 == 1:
                            pop_filler()
                        _emit_av(pend)
                        pend = cur
                    _emit_av(pend)

                    # Last hp: the big copies go to the (by now idle)
                    # ScalarE so the tail's DVE chain is shorter.
                    last = qc == NQC - 1 and hp == 2
                    ocp = nc.scalar.copy if last else nc.vector.tensor_copy
                    reg = hp // 2
                    for h in (hp, hp + 1):
                        dr, _ = dloc(h)
                        ocp(o_un[:, h, q_sl], avs[h][0:64, :])
                        nc.vector.tensor_copy(
                            dn[dr : dr + 1, reg, :], avs[h][64:65, :]
                        )
                    nc.vector.reciprocal_approx_fast(
                        out=rc[0:65, reg, :], in_=dn[0:65, reg, :]
                    )
                    rcp = nc.scalar.copy if last else nc.vector.tensor_copy
                    rcp(r16[0:65, reg, :], rc[0:65, reg, :])
                    fu = lambda q=qc, p=hp, r=r16: _emit_norm(q, p, r)
                    if qc == NQC - 1 and hp == 0:
                        # last qc: normalize heads 0,1 as early as possible
                        # so the tail drain only waits on heads 2,3
                        filler_q.insert(0, fu)
                    else:
                        filler_q.append(fu)

                # spacer so the first Wo pop lands well after its ot deps
                filler_q.append(lambda: None)
                for st in range(qc * 4, qc * 4 + 4):
                    filler_q.append(lambda s=st: _emit_wo(s))

            while filler_q:
                filler_q.pop(0)()

    nc.compile()
    return nc


def _get_nc(key):
    if key not in _cached:
        _cached[key] = _build_nc(*key)
    return _cached[key]


def _host_reference(X_sem, X_sal, mask, Wq_sem, bq_sem, Wk_sem, bk_sem, Wv,
                    bv, Wq_sal, bq_sal, Wk_sal, bk_sal, Wo, bo, gamma):
    f32 = np.float32
    scale = f32(1.0 / np.sqrt(DK))

    def heads(x):
        return x.reshape(B, S, H, DK).transpose(0, 2, 1, 3)

    def lin(x, W, b):
        return (x.reshape(B * S, D) @ np.asarray(W, f32)).reshape(B, S, D) + np.asarray(b, f32)

    Xm = np.asarray(X_sem, f32)
    Xl = np.asarray(X_sal, f32)
    Q = heads(lin(Xm, Wq_sem, bq_sem))
    K = heads(lin(Xm, Wk_sem, bk_sem))
    V = heads(lin(Xm, Wv, bv))
    Ql = heads(lin(Xl, Wq_sal, bq_sal))
    Kl = heads(lin(Xl, Wk_sal, bk_sal))
    out = np.empty((B, S, D), f32)
    for b in range(B):
        for h in range(H):
            A = (Q[b, h] @ K[b, h].T + gamma * (Ql[b, h] @ Kl[b, h].T)) * scale
            A = np.where(np.asarray(mask)[b, 0] == 0, f32(-1e9), A)
            A -= A.max(axis=-1, keepdims=True)
            np.exp(A, out=A)
            A /= A.sum(axis=-1, keepdims=True)
            out[b, :, h * DK : (h + 1) * DK] = A @ V[b, h]
    y = out.reshape(B * S, D) @ np.asarray(Wo, f32)
    return (y + np.asarray(bo, f32)).reshape(B, S, D)


def _rearrange_w(w):
    # [1024, C] -> [128, 8*C] with w_r[p, c*C + j] = w[c*128 + p, j]
    C = w.shape[1]
    return np.ascontiguousarray(
        w.reshape(8, 128, C).transpose(1, 0, 2).reshape(128, 8 * C)
    )


def _run_spmd_fast(nc, in_maps, n_cores):
    """run_bass_via_pjrt's multi-core path, but downloading each output
    array once instead of once per core (the stock helper re-gathers the
    sharded global for every core slice -- ~0.3s x 8 over the tunnel)."""
    import jax
    import numpy as _np
    from jax.sharding import Mesh, PartitionSpec
    from jax.experimental.shard_map import shard_map
    from concourse import mybir
    from concourse.bass2jax import (
        _bass_exec_p,
        install_neuronx_cc_hook,
        partition_id_tensor,
    )

    install_neuronx_cc_hook()
    partition_name = nc.partition_id_tensor.name if nc.partition_id_tensor else None
    in_names, out_names, out_avals, zero_outs = [], [], [], []
    for alloc in nc.m.functions[0].allocations:
        if not isinstance(alloc, mybir.MemoryLocationSet):
            continue
        name = alloc.memorylocations[0].name
        if alloc.kind == "ExternalInput":
            if name != partition_name:
                in_names.append(name)
        elif alloc.kind == "ExternalOutput":
            dt = mybir.dt.np(alloc.dtype)
            out_names.append(name)
            out_avals.append(jax.core.ShapedArray(tuple(alloc.tensor_shape), dt))
            zero_outs.append(_np.zeros(tuple(alloc.tensor_shape), dt))
    n_params, n_outs = len(in_names), len(out_names)
    in_names = in_names + out_names + ([partition_name] if partition_name else [])
    donate = tuple(range(n_params, n_params + n_outs))

    def _body(*args):
        operands = list(args)
        if partition_name is not None:
            operands.append(partition_id_tensor())
        return tuple(_bass_exec_p.bind(
            *operands,
            out_avals=tuple(out_avals),
            in_names=tuple(in_names),
            out_names=tuple(out_names),
            lowering_input_output_aliases=(),
            sim_require_finite=True,
            sim_require_nnan=True,
            nc=nc,
        ))

    devices = jax.devices()[:n_cores]
    mesh = Mesh(_np.asarray(devices), ("core",))
    sharded = jax.jit(
        shard_map(_body, mesh=mesh, in_specs=(PartitionSpec("core"),) * (n_params + n_outs),
                  out_specs=(PartitionSpec("core"),) * n_outs, check_rep=False),
        donate_argnums=donate, keep_unused=True,
    )
    concat_in = [
        _np.concatenate([m[in_names[i]] for m in in_maps], axis=0)
        for i in range(n_params)
    ]
    concat_zeros = [
        _np.zeros((n_cores * z.shape[0], *z.shape[1:]), z.dtype) for z in zero_outs
    ]
    out_arrs = sharded(*concat_in, *concat_zeros)
    gathered = [
        _np.asarray(a).reshape(n_cores, *out_avals[i].shape)
        for i, a in enumerate(out_arrs)
    ]
    return [
        {name: gathered[i][c] for i, name in enumerate(out_names)}
        for c in range(n_cores)
    ]


def kernel(X_sem, X_sal, mask, Wq_sem, bq_sem, Wk_sem, bk_sem, Wv, bv,
           Wq_sal, bq_sal, Wk_sal, bk_sal, Wo, bo, gamma):
    from concourse.bass_utils import run_bass_kernel_spmd

    X_sem = np.asarray(X_sem)
    X_sal = np.asarray(X_sal)
    mask = np.asarray(mask)
    f32 = np.float32
    scale = f32(1.0 / np.sqrt(DK))
    g = f32(np.asarray(gamma).reshape(()))

    wq_full = (np.asarray(Wq_sem) * scale).astype(np.float16)
    bq_full = (np.asarray(bq_sem) * scale).astype(np.float16)
    wqs_full = (np.asarray(Wq_sal) * (g * scale)).astype(np.float16)
    bqs_full = (np.asarray(bq_sal) * (g * scale)).astype(np.float16)
    wk_full = np.asarray(Wk_sem).astype(np.float16)
    bk_full = np.asarray(bk_sem).astype(np.float16)
    wks_full = np.asarray(Wk_sal).astype(np.float16)
    bks_full = np.asarray(bk_sal).astype(np.float16)
    wv_full = np.asarray(Wv).astype(np.float16)
    bv_full = np.asarray(bv).astype(np.float16)
    wo_full = np.asarray(Wo).astype(np.float16)

    if not bool(np.all(mask)):
        # Masks with zeros never occur in this problem's input spec
        # (fill: ones); handle them exactly via a host fallback.
        return _host_reference(
            X_sem, X_sal, mask, Wq_sem, bq_sem, Wk_sem, bk_sem, Wv, bv,
            Wq_sal, bq_sal, Wk_sal, bk_sal, Wo, bo, g,
        )

    with_qk_bias = bool(
        np.any(np.asarray(bq_sem)) or np.any(np.asarray(bq_sal))
        or np.any(np.asarray(bk_sem)) or np.any(np.asarray(bk_sal))
    )
    with_v_bias = bool(np.any(np.asarray(bv)))

    nc = _get_nc((with_qk_bias, with_v_bias))

    xt = []
    for b in range(B):
        xt.append((
            np.ascontiguousarray(X_sem[b].T.astype(np.float16)),
            np.ascontiguousarray(X_sal[b].T.astype(np.float16)),
        ))

    in_maps = []
    for c in range(N_CORES):
        b, hg = c // HG, c % HG
        blk = slice(hg * DKC, (hg + 1) * DKC)
        m = {
            "xt_sem": xt[b][0],
            "xt_sal": xt[b][1],
            "wq": _rearrange_w(wq_full[:, blk]),
            "wk": _rearrange_w(wk_full[:, blk]),
            "wqs": _rearrange_w(wqs_full[:, blk]),
            "wks": _rearrange_w(wks_full[:, blk]),
            "wv": _rearrange_w(wv_full[:, blk]),
            "wo": np.ascontiguousarray(
                wo_full[blk].reshape(2, 128, D).transpose(1, 0, 2).reshape(128, 2 * D)
            ),
        }
        if with_qk_bias:
            m["bqk"] = np.concatenate(
                [bq_full[blk], bqs_full[blk], bk_full[blk], bks_full[blk]]
            ).reshape(1, 4 * DKC)
        if with_v_bias:
            m["bv"] = bv_full[blk].reshape(1, DKC)
        in_maps.append(m)

    try:
        results = _run_spmd_fast(nc, in_maps, N_CORES)
    except Exception:
        results = run_bass_kernel_spmd(
            nc, in_maps, core_ids=list(range(N_CORES))
        ).results

    out = np.zeros((B, S, D), dtype=f32)
    for c in range(N_CORES):
        out[c // HG] += results[c]["out"].astype(f32)
    out += np.asarray(bo).astype(f32)
    return out
